# revision 42
# baseline (speedup 1.0000x reference)
"""GQA causal attention with RoPE, distributed over 8 trn2 NeuronCores.

Sharding: 4-way data parallel over batch x 2-way tensor parallel over heads.
Core c = 2*b + t handles batch b with query heads [t*8, (t+1)*8) and KV heads
[t*2, (t+1)*2).  Each core computes a row-sharded out_proj partial; the pair
partials are summed on the host during unsharding.

On-chip algorithm (per core, bf16 matmuls / fp32 softmax):
  1. QKV projections from host-pretransposed xT (feature-major).
  2. RoPE applied token-major straight out of PSUM, cast to bf16, and
     PE-transposed into feature-major qT / kT for the scores matmul.
  3. Scores computed TRANSPOSED (scoresT[k_tok, q_tok]) so no probs transpose
     is needed: exp on ScalarE, row sums via ones-matmul on PE, AV matmul
     consumes probsT directly, normalization happens once on the attention
     output (pav * broadcast(1/sums)).
  4. Causality: blocks with ki > qi are never computed; the diagonal block is
     masked with a precomputed upper-triangular 0/1 mask after exp.
  5. out_proj from feature-major attnT with wo as the moving operand.
"""

import math
import sys

sys.path.insert(0, "/opt/trn_rl_repo")

import ml_dtypes
import numpy as np

import concourse.bacc as bacc
import concourse.mybir as mybir
import concourse.tile as tile
from concourse.bass import _add_dep_helper
from concourse.bass_utils import run_bass_kernel_spmd
from concourse.masks import make_identity, make_upper_triangular

B, S, HID = 4, 1024, 2048
H, KV, D = 16, 4, 128
P = 128
TP = 2                  # tensor-parallel ways (head split)
HL = H // TP            # 8 query heads per core
KVL = KV // TP          # 2 kv heads per core
QD = HL * D             # 1024
KD = KVL * D            # 256
SC = S // P             # 8 token chunks
KC = HID // P           # 16 hidden chunks
NCORES = 8
BF = mybir.dt.bfloat16
F32 = mybir.dt.float32
Exp = mybir.ActivationFunctionType.Exp

_NC_CACHE = {}


def _ensure_ntff_hook():
    """The agent image's antenv lacks axon_hooks, so bass_utils' trace=True
    path can't find the NTFF profile hook trn_boot would have registered.
    Recreate the module and register the ctypes-based hook ourselves."""
    try:
        from antenv.axon_hooks import get_axon_ntff_profile_hook  # noqa: F401
        return
    except ImportError:
        pass
    import types

    import antenv

    mod = types.ModuleType("antenv.axon_hooks")
    _state = {"hook": None}
    mod.set_axon_ntff_profile_hook = lambda h: _state.__setitem__("hook", h)
    mod.get_axon_ntff_profile_hook = lambda: _state["hook"]
    sys.modules["antenv.axon_hooks"] = mod
    antenv.axon_hooks = mod
    try:
        from trn_agent_boot.trn_boot import _ntff_profile_via_ctypes

        hook = _ntff_profile_via_ctypes("/opt/axon/libaxon_pjrt.so")
        if hook is not None:
            mod.set_axon_ntff_profile_hook(hook)
    except Exception as e:  # pragma: no cover
        print(f"NTFF hook registration failed: {e}", file=sys.stderr)


def _pieces(start, end, step=512):
    """Split [start, end) into spans of at most `step`, aligned so no span
    crosses a `step` boundary (PSUM: one bank per matmul)."""
    out = []
    a = start
    while a < end:
        b = min((a // step + 1) * step, end)
        out.append((a, b))
        a = b
    return out


def build_nc(dbg=False):
    nc = bacc.Bacc("TRN2", target_bir_lowering=False, debug=False,
                   num_devices=NCORES)
    dbg_outs = {}
    if dbg:
        dbg_outs["qT"] = nc.declare_dram_parameter(
            "dbg_qT", [P, HL * S], BF, isOutput=True)
        dbg_outs["kT"] = nc.declare_dram_parameter(
            "dbg_kT", [P, KVL * S], BF, isOutput=True)
        dbg_outs["v"] = nc.declare_dram_parameter(
            "dbg_v", [P, SC * KD], BF, isOutput=True)
        dbg_outs["probsT"] = nc.declare_dram_parameter(
            "dbg_probsT", [P, SC * S], BF, isOutput=True)
        dbg_outs["attnT"] = nc.declare_dram_parameter(
            "dbg_attnT", [P, HL * S], BF, isOutput=True)

    QKVD = QD + 2 * KD          # 1536 = q 1024 | k 256 | v 256
    xT = nc.declare_dram_parameter("xT", [HID, S], BF, isOutput=False)
    wqkv = nc.declare_dram_parameter("wqkv", [HID, QKVD], BF, isOutput=False)
    wo = nc.declare_dram_parameter("wo", [QD, HID], BF, isOutput=False)
    cos_t = nc.declare_dram_parameter("cos_t", [S, D], BF, isOutput=False)
    sin_t = nc.declare_dram_parameter("sin_t", [S, D], BF, isOutput=False)
    out = nc.declare_dram_parameter("out", [S, HID], BF, isOutput=True)

    with tile.TileContext(nc) as tc:
        with (
            tc.tile_pool(name="consts", bufs=1) as cpool,
            tc.tile_pool(name="wpool", bufs=1) as wpool,
            tc.tile_pool(name="qkvpool", bufs=1) as qkvpool,
        ):
            ident = cpool.tile([P, P], BF)
            make_identity(nc, ident[:, :])
            utmask = cpool.tile([P, P], BF)
            make_upper_triangular(nc, utmask[:, :], val=1.0, diag=True)
            ones_mat = cpool.tile([P, P], BF)
            nc.vector.memset(ones_mat[:, :], 1.0)

            sb_wo = wpool.tile([P, HL, HID], BF)

            sb_qT = qkvpool.tile([P, HL, S], BF)      # feature-major q
            sb_kT = qkvpool.tile([P, KVL, S], BF)     # feature-major k
            sb_v = qkvpool.tile([P, SC, KD], BF)      # token-major v
            sb_attnT = qkvpool.tile([P, HL, S], BF)   # feature-major attn out

            # ---------------- Phase A: projections + RoPE -----------------
            with (
                tc.tile_pool(name="proj", bufs=1) as projpool,
                tc.tile_pool(name="rope", bufs=4) as ropepool,
                tc.tile_pool(name="ps_q", bufs=2, space="PSUM") as ps_q,
                tc.tile_pool(name="ps_t", bufs=2, space="PSUM") as ps_t,
            ):
                # split loads per k-chunk so the first matmuls can start as
                # soon as chunk 0 lands instead of after the full 10 MB
                sb_xT = projpool.tile([P, KC, S], BF)
                sb_wqkv = projpool.tile([P, KC, QKVD], BF)
                xT_r = xT.rearrange("(c p) s -> p c s", p=P)
                wqkv_r = wqkv.rearrange("(c p) n -> p c n", p=P)
                for c in range(KC):
                    nc.sync.dma_start(out=sb_wqkv[:, c, :], in_=wqkv_r[:, c, :])
                    nc.sync.dma_start(out=sb_xT[:, c, :], in_=xT_r[:, c, :])
                sb_ck = projpool.tile([P, SC, D], BF)
                nc.sync.dma_start(
                    out=sb_ck[:, :, :],
                    in_=cos_t.rearrange("(m p) d -> p m d", p=P),
                )
                sb_sk = projpool.tile([P, SC, D], BF)
                nc.sync.dma_start(
                    out=sb_sk[:, :, :],
                    in_=sin_t.rearrange("(m p) d -> p m d", p=P),
                )
                # wo is only needed in phase C: delay its (4 MB) load until
                # mid-phase-A so it doesn't compete with xT/wqkv on HBM
                wo_dma = nc.sync.dma_start(
                    out=sb_wo[:, :, :],
                    in_=wo.rearrange("(c p) n -> p c n", p=P),
                )

                HALF = D // 2

                def rope_block(psrc, lo, nh, sb_cos, sb_sin, m):
                    """RoPE `nh` consecutive heads of PSUM (cols [lo, lo+nh*D))
                    in one batched op per step, via free-dim-broadcast cos/sin
                    APs. Returns a bf16 SBUF tile [P, nh*D]."""
                    t1 = ropepool.tile([P, nh, D], F32, tag=f"t1_{nh}")
                    ro = ropepool.tile([P, nh * D], BF, tag=f"ro_{nh}")
                    src = psrc[:, lo:lo + nh * D].rearrange(
                        "p (h d) -> p h d", h=nh)
                    sin_lo = sb_sin[:, m:m + 1, 0:HALF].broadcast_to(
                        [P, nh, HALF])
                    sin_hi = sb_sin[:, m:m + 1, HALF:D].broadcast_to(
                        [P, nh, HALF])
                    cos_b = sb_cos[:, m:m + 1, :].broadcast_to([P, nh, D])
                    # rot_half * sin (sin table pre-negated on first half)
                    nc.vector.tensor_mul(t1[:, :, 0:HALF], src[:, :, HALF:D],
                                         sin_lo)
                    nc.vector.tensor_mul(t1[:, :, HALF:D], src[:, :, 0:HALF],
                                         sin_hi)
                    ror = ro[:, :].rearrange("p (h d) -> p h d", h=nh)
                    # ro = src*cos + t1
                    nc.vector.tensor_mul(ror, src, cos_b)
                    nc.vector.tensor_add(ror, ror, t1[:, :, :])
                    return ro

                def transpose_pack(ro, nh, dst, on_vector=False):
                    """PE-transpose nh [P, D] chunks of ro into one packed
                    PSUM tile (single bank-clearing start), then one ScalarE
                    copy into the feature-major destination AP."""
                    pt_full = ps_t.tile([P, 4 * P], BF, tag="pt")
                    pt = pt_full[:, 0:nh * P]
                    for i in range(nh):
                        nc.tensor.matmul(pt[:, i * P:(i + 1) * P],
                                         ro[:, i * D:(i + 1) * D],
                                         ident[:, :], is_transpose=True,
                                         start=(i == 0), stop=(i == nh - 1))
                    src = pt[:, :].rearrange("p (h t) -> p h t", h=nh)
                    if on_vector:
                        nc.vector.tensor_copy(dst, src)
                    else:
                        nc.scalar.copy(dst, src)

                def proj_mms(pqkv, m, k):
                    st, sp = (k == 0), (k == KC - 1)
                    lhsT = sb_xT[:, k, m * P:(m + 1) * P]
                    for n in range(QKVD // 512):
                        mm = nc.tensor.matmul(
                            pqkv[:, n * 512:(n + 1) * 512], lhsT,
                            sb_wqkv[:, k, n * 512:(n + 1) * 512],
                            start=st, stop=sp)
                    return mm

                def finish_m(pqkv, m):
                    ms = slice(m * P, (m + 1) * P)
                    # K first: phase B's first dependency is kT complete.
                    # Last chunk's copies go on DVE so ScalarE is free for
                    # phase B's first exps.
                    ov = (m == SC - 1)
                    k_ro = rope_block(pqkv, QD, KVL, sb_ck, sb_sk, m)
                    q_ro = rope_block(pqkv, 0, HL, sb_ck, sb_sk, m)
                    transpose_pack(k_ro, KVL, sb_kT[:, :, ms], on_vector=ov)
                    transpose_pack(q_ro[:, 0:4 * D], 4, sb_qT[:, 0:4, ms],
                                   on_vector=ov)
                    transpose_pack(q_ro[:, 4 * D:8 * D], 4, sb_qT[:, 4:8, ms],
                                   on_vector=ov)
                    nc.scalar.copy(sb_v[:, m, :],
                                   pqkv[:, QD + KD:QD + 2 * KD])

                # m=0 and m=1 interleaved per k-chunk: during the input DMA
                # ramp each arriving chunk feeds 2x the matmul work
                pqkv0 = ps_q.tile([P, QKVD], F32, tag="pqkv")
                pqkv1 = ps_q.tile([P, QKVD], F32, tag="pqkv")
                for k in range(KC):
                    proj_mms(pqkv0, 0, k)
                    proj_mms(pqkv1, 1, k)
                finish_m(pqkv0, 0)
                finish_m(pqkv1, 1)
                for m in range(2, SC):
                    pqkv = ps_q.tile([P, QKVD], F32, tag="pqkv")
                    for k in range(KC):
                        mm = proj_mms(pqkv, m, k)
                    if m == 2:
                        # release the wo load only once the input streaming
                        # has mostly drained
                        _add_dep_helper(wo_dma.ins, mm.ins,
                                        reason="delay wo load past input ramp")
                    finish_m(pqkv, m)

            # ---------------- Phase B: causal attention -------------------
            with (
                tc.tile_pool(name="attn", bufs=3) as attnpool,
                tc.tile_pool(name="norm", bufs=2) as normpool,
                tc.tile_pool(name="ps_sc", bufs=2, space="PSUM") as ps_sc,
                tc.tile_pool(name="ps_av", bufs=2, space="PSUM") as ps_av,
                tc.tile_pool(name="ps_bc", bufs=1, space="PSUM") as ps_bc,
            ):
                if dbg:
                    nc.sync.dma_start(out=dbg_outs["qT"][:, :],
                                      in_=sb_qT[:, :, :])
                    nc.sync.dma_start(out=dbg_outs["kT"][:, :],
                                      in_=sb_kT[:, :, :])
                    nc.sync.dma_start(out=dbg_outs["v"][:, :],
                                      in_=sb_v[:, :, :])

                def make_head(h, g, probsT, acc, pav):
                    def av(ki):
                        st, sp = (ki == 0), (ki == SC - 1)
                        for (a, b) in _pieces(ki * P, S):
                            nc.tensor.matmul(pav[:, a:b],
                                             sb_v[:, ki, g * D:(g + 1) * D],
                                             probsT[:, ki, a:b],
                                             start=st, stop=sp)

                    def finalize():
                        av(SC - 1)
                        # ones-matrix matmul = column sums already broadcast
                        # across all partitions, straight into PSUM — no
                        # ScalarE copy / GpSimd broadcast hops
                        psbc = ps_bc.tile([P, S], F32, tag="psbc")
                        for (a, b) in _pieces(0, S):
                            nc.tensor.matmul(psbc[:, a:b], ones_mat[:, :],
                                             acc[:, a:b],
                                             start=True, stop=True)
                        rbc = normpool.tile([P, S], F32, tag="rbc")
                        nc.vector.reciprocal_approx_fast(rbc[:, :],
                                                         psbc[:, :])
                        nc.vector.tensor_mul(sb_attnT[:, h, :], pav[:, :],
                                             rbc[:, :])

                    return av, finalize

                pending = [None]
                for h in range(HL):
                    g = h // (HL // KVL)
                    probsT = attnpool.tile([P, SC, S], BF, tag="probsT")
                    if dbg and h == 0:
                        dbg_probsT = probsT
                    acc = attnpool.tile([P, S], BF, tag="acc")
                    pav = ps_av.tile([P, S], F32, tag="pav")
                    av, finalize = make_head(h, g, probsT, acc, pav)

                    for ki in range(SC):
                        q0 = ki * P
                        kslice = slice(q0, q0 + P)
                        for (a, b) in _pieces(q0, S):
                            psc = ps_sc.tile([P, 512], F32, tag="psc")
                            nc.tensor.matmul(psc[:, 0:b - a],
                                             sb_kT[:, g, kslice],
                                             sb_qT[:, h, a:b],
                                             start=True, stop=True)
                            nc.scalar.activation(probsT[:, ki, a:b],
                                                 psc[:, 0:b - a], Exp,
                                                 scale=float(1 / math.sqrt(D)))
                        # mask strictly-below-diagonal of the diag block
                        nc.vector.tensor_mul(probsT[:, ki, q0:q0 + P],
                                             probsT[:, ki, q0:q0 + P],
                                             utmask[:, :])
                        # accumulate the column sums on DVE: one tile add per
                        # ki instead of 12 ones-matmuls per head on the PE
                        if ki == 0:
                            nc.vector.tensor_copy(acc[:, :], probsT[:, 0, :])
                        else:
                            nc.vector.tensor_add(acc[:, q0:], acc[:, q0:],
                                                 probsT[:, ki, q0:])
                        if ki >= 1:
                            av(ki - 1)
                        # previous head's ~5 us tail chain runs inside this
                        # head's compute instead of stalling the PE
                        if ki == 3 and pending[0] is not None:
                            pending[0]()
                            pending[0] = None
                    pending[0] = finalize
                    if dbg and h == 0:
                        nc.sync.dma_start(out=dbg_outs["probsT"][:, :],
                                          in_=dbg_probsT[:, :, :])
                pending[0]()

            if dbg:
                nc.sync.dma_start(out=dbg_outs["attnT"][:, :],
                                  in_=sb_attnT[:, :, :])

            # ---------------- Phase C: out projection ---------------------
            with (
                tc.tile_pool(name="ysb", bufs=2) as ypool,
                tc.tile_pool(name="ps_y", bufs=2, space="PSUM") as ps_y,
            ):
                for m in range(SC):
                    ms = slice(m * P, (m + 1) * P)
                    py = ps_y.tile([P, HID], F32, tag="py")
                    for k in range(HL):
                        st, sp = (k == 0), (k == HL - 1)
                        lhsT = sb_attnT[:, k, ms]
                        for n in range(HID // 512):
                            nc.tensor.matmul(py[:, n * 512:(n + 1) * 512],
                                             lhsT,
                                             sb_wo[:, k, n * 512:(n + 1) * 512],
                                             start=st, stop=sp)
                    ysb = ypool.tile([P, HID], BF, tag="ysb")
                    if m == SC - 1:
                        # split the last copy across both engines: it is the
                        # serial tail before the final store
                        nc.scalar.copy(ysb[:, 0:HID // 2], py[:, 0:HID // 2])
                        nc.vector.tensor_copy(ysb[:, HID // 2:HID],
                                              py[:, HID // 2:HID])
                    else:
                        nc.scalar.copy(ysb[:, :], py[:, :])
                    nc.sync.dma_start(out=out[ms, :], in_=ysb[:, :])

    nc.compile()
    return nc


def _get_nc():
    if "nc" not in _NC_CACHE:
        _NC_CACHE["nc"] = build_nc()
    return _NC_CACHE["nc"]


def _make_in_maps(x, cos, sin, wq, wk, wv, wo):
    bf = ml_dtypes.bfloat16
    HALF = D // 2
    sin_rot = np.concatenate([-sin[:, :HALF], sin[:, HALF:]], axis=1)
    cos_t = cos.astype(bf)
    sin_t = sin_rot.astype(bf)
    in_maps = []
    for core in range(NCORES):
        b, t = divmod(core, TP)
        wqkv = np.concatenate([
            wq[:, t * QD:(t + 1) * QD],
            wk[:, t * KD:(t + 1) * KD],
            wv[:, t * KD:(t + 1) * KD],
        ], axis=1)
        in_maps.append({
            "xT": np.ascontiguousarray(x[b].T).astype(bf),
            "wqkv": np.ascontiguousarray(wqkv).astype(bf),
            "wo": np.ascontiguousarray(wo[t * QD:(t + 1) * QD, :]).astype(bf),
            "cos_t": cos_t, "sin_t": sin_t,
        })
    return in_maps


def run(inputs, trace=False):
    if trace:
        _ensure_ntff_hook()
    nc = _get_nc()
    in_maps = _make_in_maps(
        np.asarray(inputs["x"], np.float32),
        np.asarray(inputs["cos"], np.float32),
        np.asarray(inputs["sin"], np.float32),
        np.asarray(inputs["wq"], np.float32),
        np.asarray(inputs["wk"], np.float32),
        np.asarray(inputs["wv"], np.float32),
        np.asarray(inputs["wo"], np.float32),
    )
    res = run_bass_kernel_spmd(nc, in_maps, list(range(NCORES)), trace=trace)
    outs = [np.asarray(r["out"]).astype(np.float32) for r in res.results]
    y = np.stack([outs[TP * b] + outs[TP * b + 1] for b in range(B)])
    return y, res


def kernel(**inputs):
    y, _ = run(inputs, trace=False)
    return y


# revision 43
# speedup vs baseline: 1.0223x; 1.0223x over previous
"""GQA causal attention with RoPE, distributed over 8 trn2 NeuronCores.

Sharding: 4-way data parallel over batch x 2-way tensor parallel over heads.
Core c = 2*b + t handles batch b with query heads [t*8, (t+1)*8) and KV heads
[t*2, (t+1)*2).  Each core computes a row-sharded out_proj partial; the pair
partials are summed on the host during unsharding.

On-chip algorithm (per core, bf16 matmuls / fp32 softmax):
  1. QKV projections from host-pretransposed xT (feature-major).
  2. RoPE applied token-major straight out of PSUM, cast to bf16, and
     PE-transposed into feature-major qT / kT for the scores matmul.
  3. Scores computed TRANSPOSED (scoresT[k_tok, q_tok]) so no probs transpose
     is needed: exp on ScalarE, row sums via ones-matmul on PE, AV matmul
     consumes probsT directly, normalization happens once on the attention
     output (pav * broadcast(1/sums)).
  4. Causality: blocks with ki > qi are never computed; the diagonal block is
     masked with a precomputed upper-triangular 0/1 mask after exp.
  5. out_proj from feature-major attnT with wo as the moving operand.
"""

import math
import sys

sys.path.insert(0, "/opt/trn_rl_repo")

import ml_dtypes
import numpy as np

import concourse.bacc as bacc
import concourse.mybir as mybir
import concourse.tile as tile
from concourse.bass import _add_dep_helper
from concourse.bass_utils import run_bass_kernel_spmd
from concourse.masks import make_identity, make_upper_triangular

B, S, HID = 4, 1024, 2048
H, KV, D = 16, 4, 128
P = 128
TP = 2                  # tensor-parallel ways (head split)
HL = H // TP            # 8 query heads per core
KVL = KV // TP          # 2 kv heads per core
QD = HL * D             # 1024
KD = KVL * D            # 256
SC = S // P             # 8 token chunks
KC = HID // P           # 16 hidden chunks
NCORES = 8
BF = mybir.dt.bfloat16
F32 = mybir.dt.float32
Exp = mybir.ActivationFunctionType.Exp

_NC_CACHE = {}


def _ensure_ntff_hook():
    """The agent image's antenv lacks axon_hooks, so bass_utils' trace=True
    path can't find the NTFF profile hook trn_boot would have registered.
    Recreate the module and register the ctypes-based hook ourselves."""
    try:
        from antenv.axon_hooks import get_axon_ntff_profile_hook  # noqa: F401
        return
    except ImportError:
        pass
    import types

    import antenv

    mod = types.ModuleType("antenv.axon_hooks")
    _state = {"hook": None}
    mod.set_axon_ntff_profile_hook = lambda h: _state.__setitem__("hook", h)
    mod.get_axon_ntff_profile_hook = lambda: _state["hook"]
    sys.modules["antenv.axon_hooks"] = mod
    antenv.axon_hooks = mod
    try:
        from trn_agent_boot.trn_boot import _ntff_profile_via_ctypes

        hook = _ntff_profile_via_ctypes("/opt/axon/libaxon_pjrt.so")
        if hook is not None:
            mod.set_axon_ntff_profile_hook(hook)
    except Exception as e:  # pragma: no cover
        print(f"NTFF hook registration failed: {e}", file=sys.stderr)


def _pieces(start, end, step=512):
    """Split [start, end) into spans of at most `step`, aligned so no span
    crosses a `step` boundary (PSUM: one bank per matmul)."""
    out = []
    a = start
    while a < end:
        b = min((a // step + 1) * step, end)
        out.append((a, b))
        a = b
    return out


def build_nc(dbg=False):
    nc = bacc.Bacc("TRN2", target_bir_lowering=False, debug=False,
                   num_devices=NCORES)
    dbg_outs = {}
    if dbg:
        dbg_outs["qT"] = nc.declare_dram_parameter(
            "dbg_qT", [P, HL * S], BF, isOutput=True)
        dbg_outs["kT"] = nc.declare_dram_parameter(
            "dbg_kT", [P, KVL * S], BF, isOutput=True)
        dbg_outs["v"] = nc.declare_dram_parameter(
            "dbg_v", [P, SC * KD], BF, isOutput=True)
        dbg_outs["probsT"] = nc.declare_dram_parameter(
            "dbg_probsT", [P, SC * S], BF, isOutput=True)
        dbg_outs["attnT"] = nc.declare_dram_parameter(
            "dbg_attnT", [P, HL * S], BF, isOutput=True)

    QKVD = QD + 2 * KD          # 1536 = q 1024 | k 256 | v 256
    xT = nc.declare_dram_parameter("xT", [HID, S], BF, isOutput=False)
    wqkv = nc.declare_dram_parameter("wqkv", [HID, QKVD], BF, isOutput=False)
    wo = nc.declare_dram_parameter("wo", [QD, HID], BF, isOutput=False)
    cos_t = nc.declare_dram_parameter("cos_t", [S, D], BF, isOutput=False)
    sin_t = nc.declare_dram_parameter("sin_t", [S, D], BF, isOutput=False)
    out = nc.declare_dram_parameter("out", [S, HID], BF, isOutput=True)

    with tile.TileContext(nc) as tc:
        with (
            tc.tile_pool(name="consts", bufs=1) as cpool,
            tc.tile_pool(name="wpool", bufs=1) as wpool,
            tc.tile_pool(name="qkvpool", bufs=1) as qkvpool,
        ):
            ident = cpool.tile([P, P], BF)
            make_identity(nc, ident[:, :])
            utmask = cpool.tile([P, P], BF)
            make_upper_triangular(nc, utmask[:, :], val=1.0, diag=True)
            ones_mat = cpool.tile([P, P], BF)
            nc.vector.memset(ones_mat[:, :], 1.0)

            sb_wo = wpool.tile([P, HL, HID], BF)

            sb_qT = qkvpool.tile([P, HL, S], BF)      # feature-major q
            sb_kT = qkvpool.tile([P, KVL, S], BF)     # feature-major k
            sb_v = qkvpool.tile([P, SC, KD], BF)      # token-major v
            sb_attnT = qkvpool.tile([P, HL, S], BF)   # feature-major attn out

            # ---------------- Phase A: projections + RoPE -----------------
            with (
                tc.tile_pool(name="proj", bufs=1) as projpool,
                tc.tile_pool(name="rope", bufs=4) as ropepool,
                tc.tile_pool(name="ps_q", bufs=2, space="PSUM") as ps_q,
                tc.tile_pool(name="ps_t", bufs=2, space="PSUM") as ps_t,
            ):
                # split loads per k-chunk so the first matmuls can start as
                # soon as chunk 0 lands instead of after the full 10 MB
                sb_xT = projpool.tile([P, KC, S], BF)
                sb_wqkv = projpool.tile([P, KC, QKVD], BF)
                xT_r = xT.rearrange("(c p) s -> p c s", p=P)
                wqkv_r = wqkv.rearrange("(c p) n -> p c n", p=P)
                for c in range(KC):
                    nc.sync.dma_start(out=sb_wqkv[:, c, :], in_=wqkv_r[:, c, :])
                    nc.sync.dma_start(out=sb_xT[:, c, :], in_=xT_r[:, c, :])
                sb_ck = projpool.tile([P, SC, D], BF)
                nc.sync.dma_start(
                    out=sb_ck[:, :, :],
                    in_=cos_t.rearrange("(m p) d -> p m d", p=P),
                )
                sb_sk = projpool.tile([P, SC, D], BF)
                nc.sync.dma_start(
                    out=sb_sk[:, :, :],
                    in_=sin_t.rearrange("(m p) d -> p m d", p=P),
                )
                # wo is only needed in phase C: delay its (4 MB) load until
                # mid-phase-A so it doesn't compete with xT/wqkv on HBM
                wo_dma = nc.sync.dma_start(
                    out=sb_wo[:, :, :],
                    in_=wo.rearrange("(c p) n -> p c n", p=P),
                )

                HALF = D // 2

                def rope_block(psrc, lo, nh, sb_cos, sb_sin, m):
                    """RoPE `nh` consecutive heads of PSUM (cols [lo, lo+nh*D))
                    in one batched op per step, via free-dim-broadcast cos/sin
                    APs. Returns a bf16 SBUF tile [P, nh*D]."""
                    t1 = ropepool.tile([P, nh, D], F32, tag=f"t1_{nh}")
                    ro = ropepool.tile([P, nh * D], BF, tag=f"ro_{nh}")
                    src = psrc[:, lo:lo + nh * D].rearrange(
                        "p (h d) -> p h d", h=nh)
                    sin_lo = sb_sin[:, m:m + 1, 0:HALF].broadcast_to(
                        [P, nh, HALF])
                    sin_hi = sb_sin[:, m:m + 1, HALF:D].broadcast_to(
                        [P, nh, HALF])
                    cos_b = sb_cos[:, m:m + 1, :].broadcast_to([P, nh, D])
                    # rot_half * sin (sin table pre-negated on first half)
                    nc.vector.tensor_mul(t1[:, :, 0:HALF], src[:, :, HALF:D],
                                         sin_lo)
                    nc.vector.tensor_mul(t1[:, :, HALF:D], src[:, :, 0:HALF],
                                         sin_hi)
                    ror = ro[:, :].rearrange("p (h d) -> p h d", h=nh)
                    # ro = src*cos + t1
                    nc.vector.tensor_mul(ror, src, cos_b)
                    nc.vector.tensor_add(ror, ror, t1[:, :, :])
                    return ro

                def transpose_pack(ro, nh, dst, on_vector=False):
                    """PE-transpose nh [P, D] chunks of ro into one packed
                    PSUM tile (single bank-clearing start), then one ScalarE
                    copy into the feature-major destination AP."""
                    pt_full = ps_t.tile([P, 4 * P], BF, tag="pt")
                    pt = pt_full[:, 0:nh * P]
                    for i in range(nh):
                        nc.tensor.matmul(pt[:, i * P:(i + 1) * P],
                                         ro[:, i * D:(i + 1) * D],
                                         ident[:, :], is_transpose=True,
                                         start=(i == 0), stop=(i == nh - 1))
                    src = pt[:, :].rearrange("p (h t) -> p h t", h=nh)
                    if on_vector:
                        nc.vector.tensor_copy(dst, src)
                    else:
                        nc.scalar.copy(dst, src)

                def proj_mms(pqkv, m, k):
                    st, sp = (k == 0), (k == KC - 1)
                    lhsT = sb_xT[:, k, m * P:(m + 1) * P]
                    for n in range(QKVD // 512):
                        mm = nc.tensor.matmul(
                            pqkv[:, n * 512:(n + 1) * 512], lhsT,
                            sb_wqkv[:, k, n * 512:(n + 1) * 512],
                            start=st, stop=sp)
                    return mm

                def finish_m(pqkv, m):
                    ms = slice(m * P, (m + 1) * P)
                    # K first: phase B's first dependency is kT complete.
                    # Last chunk's copies go on DVE so ScalarE is free for
                    # phase B's first exps.
                    ov = (m == SC - 1)
                    k_ro = rope_block(pqkv, QD, KVL, sb_ck, sb_sk, m)
                    transpose_pack(k_ro, KVL, sb_kT[:, :, ms], on_vector=ov)
                    q_ro = rope_block(pqkv, 0, HL, sb_ck, sb_sk, m)
                    transpose_pack(q_ro[:, 0:4 * D], 4, sb_qT[:, 0:4, ms],
                                   on_vector=ov)
                    transpose_pack(q_ro[:, 4 * D:8 * D], 4, sb_qT[:, 4:8, ms],
                                   on_vector=ov)
                    nc.scalar.copy(sb_v[:, m, :],
                                   pqkv[:, QD + KD:QD + 2 * KD])

                # m=0 and m=1 interleaved per k-chunk: during the input DMA
                # ramp each arriving chunk feeds 2x the matmul work
                pqkv0 = ps_q.tile([P, QKVD], F32, tag="pqkv")
                pqkv1 = ps_q.tile([P, QKVD], F32, tag="pqkv")
                for k in range(KC):
                    proj_mms(pqkv0, 0, k)
                    proj_mms(pqkv1, 1, k)
                finish_m(pqkv0, 0)
                finish_m(pqkv1, 1)
                for m in range(2, SC):
                    pqkv = ps_q.tile([P, QKVD], F32, tag="pqkv")
                    for k in range(KC):
                        mm = proj_mms(pqkv, m, k)
                    if m == 2:
                        # release the wo load only once the input streaming
                        # has mostly drained
                        _add_dep_helper(wo_dma.ins, mm.ins,
                                        reason="delay wo load past input ramp")
                    finish_m(pqkv, m)

            # ---------------- Phase B: causal attention -------------------
            with (
                tc.tile_pool(name="attn", bufs=3) as attnpool,
                tc.tile_pool(name="norm", bufs=2) as normpool,
                tc.tile_pool(name="ps_sc", bufs=2, space="PSUM") as ps_sc,
                tc.tile_pool(name="ps_av", bufs=2, space="PSUM") as ps_av,
                tc.tile_pool(name="ps_bc", bufs=1, space="PSUM") as ps_bc,
            ):
                if dbg:
                    nc.sync.dma_start(out=dbg_outs["qT"][:, :],
                                      in_=sb_qT[:, :, :])
                    nc.sync.dma_start(out=dbg_outs["kT"][:, :],
                                      in_=sb_kT[:, :, :])
                    nc.sync.dma_start(out=dbg_outs["v"][:, :],
                                      in_=sb_v[:, :, :])

                def make_head(h, g, probsT, acc, pav):
                    def av(ki):
                        st, sp = (ki == 0), (ki == SC - 1)
                        for (a, b) in _pieces(ki * P, S):
                            nc.tensor.matmul(pav[:, a:b],
                                             sb_v[:, ki, g * D:(g + 1) * D],
                                             probsT[:, ki, a:b],
                                             start=st, stop=sp)

                    def finalize():
                        av(SC - 1)
                        # ones-matrix matmul = column sums already broadcast
                        # across all partitions, straight into PSUM — no
                        # ScalarE copy / GpSimd broadcast hops
                        psbc = ps_bc.tile([P, S], F32, tag="psbc")
                        for (a, b) in _pieces(0, S):
                            nc.tensor.matmul(psbc[:, a:b], ones_mat[:, :],
                                             acc[:, a:b],
                                             start=True, stop=True)
                        rbc = normpool.tile([P, S], F32, tag="rbc")
                        nc.vector.reciprocal_approx_fast(rbc[:, :],
                                                         psbc[:, :])
                        nc.vector.tensor_mul(sb_attnT[:, h, :], pav[:, :],
                                             rbc[:, :])

                    return av, finalize

                pending = [None]
                for h in range(HL):
                    g = h // (HL // KVL)
                    probsT = attnpool.tile([P, SC, S], BF, tag="probsT")
                    if dbg and h == 0:
                        dbg_probsT = probsT
                    acc = attnpool.tile([P, S], BF, tag="acc")
                    pav = ps_av.tile([P, S], F32, tag="pav")
                    av, finalize = make_head(h, g, probsT, acc, pav)

                    for ki in range(SC):
                        q0 = ki * P
                        kslice = slice(q0, q0 + P)
                        for (a, b) in _pieces(q0, S):
                            psc = ps_sc.tile([P, 512], F32, tag="psc")
                            nc.tensor.matmul(psc[:, 0:b - a],
                                             sb_kT[:, g, kslice],
                                             sb_qT[:, h, a:b],
                                             start=True, stop=True)
                            nc.scalar.activation(probsT[:, ki, a:b],
                                                 psc[:, 0:b - a], Exp,
                                                 scale=float(1 / math.sqrt(D)))
                        # mask strictly-below-diagonal of the diag block
                        nc.vector.tensor_mul(probsT[:, ki, q0:q0 + P],
                                             probsT[:, ki, q0:q0 + P],
                                             utmask[:, :])
                        # accumulate the column sums on DVE: one tile add per
                        # ki instead of 12 ones-matmuls per head on the PE
                        if ki == 0:
                            nc.vector.tensor_copy(acc[:, :], probsT[:, 0, :])
                        else:
                            nc.vector.tensor_add(acc[:, q0:], acc[:, q0:],
                                                 probsT[:, ki, q0:])
                        if ki >= 1:
                            av(ki - 1)
                        # previous head's ~5 us tail chain runs inside this
                        # head's compute instead of stalling the PE
                        if ki == 3 and pending[0] is not None:
                            pending[0]()
                            pending[0] = None
                    pending[0] = finalize
                    if dbg and h == 0:
                        nc.sync.dma_start(out=dbg_outs["probsT"][:, :],
                                          in_=dbg_probsT[:, :, :])
                pending[0]()

            if dbg:
                nc.sync.dma_start(out=dbg_outs["attnT"][:, :],
                                  in_=sb_attnT[:, :, :])

            # ---------------- Phase C: out projection ---------------------
            with (
                tc.tile_pool(name="ysb", bufs=2) as ypool,
                tc.tile_pool(name="ps_y", bufs=2, space="PSUM") as ps_y,
            ):
                for m in range(SC):
                    ms = slice(m * P, (m + 1) * P)
                    py = ps_y.tile([P, HID], F32, tag="py")
                    for k in range(HL):
                        st, sp = (k == 0), (k == HL - 1)
                        lhsT = sb_attnT[:, k, ms]
                        for n in range(HID // 512):
                            nc.tensor.matmul(py[:, n * 512:(n + 1) * 512],
                                             lhsT,
                                             sb_wo[:, k, n * 512:(n + 1) * 512],
                                             start=st, stop=sp)
                    ysb = ypool.tile([P, HID], BF, tag="ysb")
                    if m == SC - 1:
                        # split the last copy across both engines: it is the
                        # serial tail before the final store
                        nc.scalar.copy(ysb[:, 0:HID // 2], py[:, 0:HID // 2])
                        nc.vector.tensor_copy(ysb[:, HID // 2:HID],
                                              py[:, HID // 2:HID])
                    else:
                        nc.scalar.copy(ysb[:, :], py[:, :])
                    nc.sync.dma_start(out=out[ms, :], in_=ysb[:, :])

    nc.compile()
    return nc


def _get_nc():
    if "nc" not in _NC_CACHE:
        _NC_CACHE["nc"] = build_nc()
    return _NC_CACHE["nc"]


def _make_in_maps(x, cos, sin, wq, wk, wv, wo):
    bf = ml_dtypes.bfloat16
    HALF = D // 2
    sin_rot = np.concatenate([-sin[:, :HALF], sin[:, HALF:]], axis=1)
    cos_t = cos.astype(bf)
    sin_t = sin_rot.astype(bf)
    in_maps = []
    for core in range(NCORES):
        b, t = divmod(core, TP)
        wqkv = np.concatenate([
            wq[:, t * QD:(t + 1) * QD],
            wk[:, t * KD:(t + 1) * KD],
            wv[:, t * KD:(t + 1) * KD],
        ], axis=1)
        in_maps.append({
            "xT": np.ascontiguousarray(x[b].T).astype(bf),
            "wqkv": np.ascontiguousarray(wqkv).astype(bf),
            "wo": np.ascontiguousarray(wo[t * QD:(t + 1) * QD, :]).astype(bf),
            "cos_t": cos_t, "sin_t": sin_t,
        })
    return in_maps


def run(inputs, trace=False):
    if trace:
        _ensure_ntff_hook()
    nc = _get_nc()
    in_maps = _make_in_maps(
        np.asarray(inputs["x"], np.float32),
        np.asarray(inputs["cos"], np.float32),
        np.asarray(inputs["sin"], np.float32),
        np.asarray(inputs["wq"], np.float32),
        np.asarray(inputs["wk"], np.float32),
        np.asarray(inputs["wv"], np.float32),
        np.asarray(inputs["wo"], np.float32),
    )
    res = run_bass_kernel_spmd(nc, in_maps, list(range(NCORES)), trace=trace)
    outs = [np.asarray(r["out"]).astype(np.float32) for r in res.results]
    y = np.stack([outs[TP * b] + outs[TP * b + 1] for b in range(B)])
    return y, res


def kernel(**inputs):
    y, _ = run(inputs, trace=False)
    return y
